# revision 17
# baseline (speedup 1.0000x reference)
"""DiffPool batched-graph layer on 8 TRN2 NeuronCores.

Decomposition: the 20 graphs are independent (edges and the assignment mask
are block-diagonal per graph).  Each graph's work reduces to small dense
matmuls against its [1000,1000] adjacency-count matrix M (built on host from
src/dst via bincount):

  per graph g (nodes X = h[g*1000:(g+1)*1000], M[s,d] = #edges s->d):
    c    = (M^T X) * inv_cnt[d]            (mean aggregation)
    x    = [X | c]                          [1000, 256]
    z_f  = x @ W_feat ;  feat = relu(z_f / max(||z_f||, 1e-12))
    n2   = rowsum(x * (x @ G)),  G = W_pool W_pool^T   (== ||x@W_pool||^2)
    z_p  = x @ W_pool[:, blk] ;  a = relu(z_p) / max(sqrt(n2), 1e-12)
    p    = softmax(a) over the 50 in-block columns   (exactly equals the
           reference's masked softmax: all out-of-block terms cancel)
    a_sT = p^T M  (== (M^T p)^T == segment_sum(p[src], dst)^T)
    h_pool_blk = p^T feat ;  adj_blk = p^T a_s
Outputs are block-diagonal; host scatters blocks into the full outputs.

Sharding: whole graphs to cores, 3 slots/core (20 real + 4 zero-pad), no
collectives needed.

Engine split (v2, after profiling): PE streams the adjacency twice
(h-stationary for cT, p-stationary for a_sT) plus the small z matmuls and
transposes; DVE does every copy/scale/reduce with fused multiply+relu
tensor_scalar ops; ACT runs exactly one batched Sqrt [125,16] and eight
consecutive Exps per graph so its function table is only reloaded twice per
graph (table thrash was 63us in v1).
"""

import sys

sys.path.insert(0, "/opt/trn_rl_repo")

import numpy as np

import concourse.bass as bass
import concourse.bacc as bacc
import concourse.mybir as mybir
import concourse.tile as tile
import concourse.bass_utils as bass_utils

F32 = mybir.dt.float32
AF = mybir.ActivationFunctionType
ALU = mybir.AluOpType
AX = mybir.AxisListType

N_PER = 1000      # nodes per graph
B = 20            # graphs
K_PER = 50        # clusters per graph
D = 128           # feature dim
N = N_PER * B
ASSIGN = K_PER * B
NCORES = 8
GPC = 3           # graph slots per core
CH = 8            # k-chunks per graph
CP = 125          # chunk size (8*125 = 1000)

_CACHE: dict = {}


def _emit(tc: "tile.TileContext", dram: dict, with_bias: bool, phases: int = 3):
    nc = tc.nc
    import contextlib

    ctx = contextlib.ExitStack()
    with ctx:
        wpool = ctx.enter_context(tc.tile_pool(name="wpool", bufs=1))
        # --- constants / weights, loaded once ---
        wf_sb = wpool.tile([128, 2 * D], F32)          # W_feat k-chunks side by side
        gm_sb = wpool.tile([128, 2 * 256], F32)        # G k-chunks
        id_sb = wpool.tile([128, 128], F32)
        for k in range(2):
            nc.sync.dma_start(wf_sb[:, k * D:(k + 1) * D], dram["wf"][k * 128:(k + 1) * 128, :])
            nc.sync.dma_start(gm_sb[:, k * 256:(k + 1) * 256], dram["gm"][k * 128:(k + 1) * 128, :])
        nc.sync.dma_start(id_sb[:, :], dram["ident"][:, :])
        if with_bias:
            ones_sb = wpool.tile([1, 128], F32)
            nc.gpsimd.memset(ones_sb[:, :], 1.0)
            bias_sb = wpool.tile([1, D + 256], F32)    # [b_feat | u], u = 2*W_pool@b_pool
            nc.sync.dma_start(bias_sb[:, :], dram["bvec"][:, :])

        pools = {}
        for name, bufs in [("af", 2), ("hsb", 2), ("xt", 2),
                           ("icnt", 2), ("wpb", 2), ("feat", 2), ("psb", 2),
                           ("zfs", 2), ("zps", 2), ("n2", 2),
                           ("ast", 2), ("assb", 2), ("scr", 2), ("sml", 3),
                           ("outs", 2)]:
            pools[name] = ctx.enter_context(tc.tile_pool(name=name, bufs=bufs))

        for j in range(GPC):
            # ---------------- loads ----------------
            a_sb = pools["af"].tile([128, CH * N_PER], F32, name=f"a_{j}", tag="a")
            h_sb = pools["hsb"].tile([128, CH * D], F32, name=f"h_{j}", tag="h")
            icnt_sb = pools["icnt"].tile([128, N_PER], F32, name=f"icnt_{j}", tag="icnt")
            wpb_sb = pools["wpb"].tile([128, 2 * K_PER], F32, name=f"wpb_{j}", tag="wpb")
            for k in range(CH):
                nc.sync.dma_start(a_sb[:CP, k * N_PER:(k + 1) * N_PER], dram["af"][j, k])
                nc.sync.dma_start(h_sb[:CP, k * D:(k + 1) * D], dram["h_in"][j, k])
            nc.sync.dma_start(icnt_sb[:, :], dram["icnt"][j])
            for k in range(2):
                nc.sync.dma_start(wpb_sb[:, k * K_PER:(k + 1) * K_PER],
                                  dram["wpb"][j, k * 128:(k + 1) * 128, :])
            if with_bias:
                bpb_sb = pools["wpb"].tile([1, K_PER], F32, name=f"bpb_{j}", tag="bpb")
                nc.sync.dma_start(bpb_sb[:, :], dram["bpb"][j])

            xt0 = pools["xt"].tile([128, N_PER], F32, name=f"xt0_{j}", tag="xt0")
            xt1 = pools["xt"].tile([128, N_PER], F32, name=f"xt1_{j}", tag="xt1")
            if phases < 1:
                continue

            # ---------------- phase 1: cT and xT ----------------
            with tc.tile_pool(name=f"ps1_{j}", bufs=1, space="PSUM") as ps1:
                # hT via PE transpose -> xt0
                for k in range(CH):
                    tp = ps1.tile([128, CP], F32, name=f"tp_{j}_{k}", tag="tp", bufs=2)
                    nc.tensor.transpose(tp[:, :], h_sb[:CP, k * D:(k + 1) * D], id_sb[:CP, :CP])
                    nc.vector.tensor_copy(xt0[:, k * CP:(k + 1) * CP], tp[:, :])
                # cT = sum_k h_k^T @ A_k  (h chunks stationary, A streams)
                ct_ps = [ps1.tile([128, 500], F32, name=f"ct{t}_{j}", tag=f"ct{t}")
                         for t in range(2)]
                for k in range(CH):
                    for t in range(2):
                        nc.tensor.matmul(
                            ct_ps[t][:, :],
                            h_sb[:CP, k * D:(k + 1) * D],
                            a_sb[:CP, k * N_PER + t * 500: k * N_PER + (t + 1) * 500],
                            start=(k == 0), stop=(k == CH - 1))
                # xt1 = cT * inv_cnt (host pre-broadcast inv_cnt rows)
                for t in range(2):
                    nc.vector.tensor_mul(
                        xt1[:, t * 500:(t + 1) * 500], ct_ps[t][:, :],
                        icnt_sb[:, t * 500:(t + 1) * 500])

            feat_sb = pools["feat"].tile([128, CH * D], F32, name=f"feat_{j}", tag="feat")
            p_sb = pools["psb"].tile([128, CH * K_PER], F32, name=f"p_{j}", tag="p")
            zfs = pools["zfs"].tile([128, CH * D], F32, name=f"zfs_{j}", tag="zfs")
            zps = pools["zps"].tile([128, CH * K_PER], F32, name=f"zps_{j}", tag="zps")
            n2all = pools["n2"].tile([128, 16], F32, name=f"n2all_{j}", tag="n2all")
            inv = pools["n2"].tile([128, 16], F32, name=f"inv_{j}", tag="inv")
            if phases < 2:
                continue

            # ---- phase 2A: z matmuls; stage z and norm^2 in SBUF (PE + DVE) ----
            with tc.tile_pool(name=f"ps2_{j}", bufs=2, space="PSUM") as ps2:
                for m in range(CH):
                    zf_ps = ps2.tile([CP, D], F32, name=f"zf_{j}_{m}", tag="zf")
                    q_ps = ps2.tile([CP, 256], F32, name=f"q_{j}_{m}", tag="q")
                    zp_ps = ps2.tile([CP, K_PER], F32, name=f"zp_{j}_{m}", tag="zp")
                    ms = slice(m * CP, (m + 1) * CP)
                    for k, xt in enumerate((xt0, xt1)):
                        last = (k == 1) and not with_bias
                        nc.tensor.matmul(zf_ps[:, :], xt[:, ms], wf_sb[:, k * D:(k + 1) * D],
                                         start=(k == 0), stop=last)
                        nc.tensor.matmul(q_ps[:, :], xt[:, ms], gm_sb[:, k * 256:(k + 1) * 256],
                                         start=(k == 0), stop=last)
                        nc.tensor.matmul(zp_ps[:, :], xt[:, ms], wpb_sb[:, k * K_PER:(k + 1) * K_PER],
                                         start=(k == 0), stop=last)
                    if with_bias:
                        nc.tensor.matmul(zf_ps[:, :], ones_sb[:1, :CP], bias_sb[:1, :D],
                                         start=False, stop=True)
                        nc.tensor.matmul(q_ps[:, :], ones_sb[:1, :CP], bias_sb[:1, D:D + 256],
                                         start=False, stop=True)
                        nc.tensor.matmul(zp_ps[:, :], ones_sb[:1, :CP], bpb_sb[:1, :],
                                         start=False, stop=True)
                    # c chunk in node-major via PE transpose of xt1 slice
                    c_ps = ps2.tile([CP, D], F32, name=f"c_{j}_{m}", tag="c")
                    nc.tensor.transpose(c_ps[:, :], xt1[:, ms], id_sb[:, :])
                    c_sb = pools["scr"].tile([CP, D], F32, name=f"csb_{j}_{m}", tag="csb")
                    nc.vector.tensor_copy(c_sb[:, :], c_ps[:, :])
                    # n2_pool = rowsum(x * q) (+ ||b_pool||^2)
                    scr = pools["scr"].tile([CP, 256], F32, name=f"scr_{j}_{m}", tag="scr")
                    nc.vector.tensor_mul(scr[:, :D], h_sb[:CP, m * D:(m + 1) * D], q_ps[:, :D])
                    nc.vector.tensor_mul(scr[:, D:256], c_sb[:, :], q_ps[:, D:256])
                    nc.vector.reduce_sum(n2all[:CP, m:m + 1], scr[:, :], axis=AX.X)
                    # stage zf / zp to SBUF; n2_feat = rowsum(zf^2)
                    fs = slice(m * D, (m + 1) * D)
                    nc.vector.tensor_copy(zfs[:CP, fs], zf_ps[:, :])
                    nc.vector.tensor_copy(zps[:CP, m * K_PER:(m + 1) * K_PER], zp_ps[:, :])
                    sq = pools["scr"].tile([CP, D], F32, name=f"sq_{j}_{m}", tag="sq")
                    nc.vector.tensor_mul(sq[:, :], zfs[:CP, fs], zfs[:CP, fs])
                    nc.vector.reduce_sum(n2all[:CP, 8 + m:9 + m], sq[:, :], axis=AX.X)
            if with_bias:
                nc.vector.tensor_scalar_add(n2all[:CP, 0:8], n2all[:CP, 0:8],
                                            float(dram["b2"]))

            # ---- phase 2B: one batched sqrt + clamp + reciprocal ----
            nrm = pools["n2"].tile([128, 16], F32, name=f"nrm_{j}", tag="nrm")
            nc.scalar.sqrt(nrm[:CP, :], n2all[:CP, :])
            nc.vector.tensor_scalar_max(nrm[:CP, :], nrm[:CP, :], 1e-12)
            nc.vector.reciprocal(inv[:CP, :], nrm[:CP, :])

            # ---- phase 2C: assign softmax + feat scaling (DVE + consecutive Exp) ----
            for m in range(CH):
                fs = slice(m * D, (m + 1) * D)
                ks = slice(m * K_PER, (m + 1) * K_PER)
                asg = pools["scr"].tile([CP, K_PER], F32, name=f"asg_{j}_{m}", tag="asg")
                nc.vector.tensor_scalar(asg[:, :], zps[:CP, ks], inv[:CP, m:m + 1], 0.0,
                                        op0=ALU.mult, op1=ALU.max)
                nmax = pools["sml"].tile([CP, 1], F32, name=f"nmax_{j}_{m}", tag="nmax")
                nc.vector.reduce_max(nmax[:, :], asg[:, :], axis=AX.X, negate=True)
                es = pools["scr"].tile([CP, K_PER], F32, name=f"es_{j}_{m}", tag="es")
                ssum = pools["sml"].tile([CP, 1], F32, name=f"ssum_{j}_{m}", tag="ssum")
                nc.scalar.activation(es[:, :], asg[:, :], AF.Exp, bias=nmax[:, :],
                                     accum_out=ssum[:, :])
                sinv = pools["sml"].tile([CP, 1], F32, name=f"sinv_{j}_{m}", tag="sinv")
                nc.vector.reciprocal(sinv[:, :], ssum[:, :])
                nc.vector.tensor_scalar_mul(p_sb[:CP, ks], es[:, :], sinv[:, :])
                nc.vector.tensor_scalar(feat_sb[:CP, fs], zfs[:CP, fs],
                                        inv[:CP, 8 + m:9 + m], 0.0,
                                        op0=ALU.mult, op1=ALU.max)

            # ---------------- phase 3: a_s, h_pool, adj ----------------
            if phases < 3:
                continue
            ast_sb = pools["ast"].tile([K_PER, N_PER], F32, name=f"ast_{j}", tag="ast")
            as_sb = pools["assb"].tile([128, CH * K_PER], F32, name=f"as_{j}", tag="as")
            adj_sb = pools["outs"].tile([K_PER, K_PER], F32, name=f"adj_{j}", tag="adj")
            hp_sb = pools["outs"].tile([K_PER, D], F32, name=f"hp_{j}", tag="hp")
            with tc.tile_pool(name=f"ps3_{j}", bufs=1, space="PSUM") as ps3:
                ast_ps = [ps3.tile([K_PER, 500], F32, name=f"ast{t}_{j}", tag=f"ast{t}")
                          for t in range(2)]
                hp_ps = ps3.tile([K_PER, D], F32, name=f"hp_{j}", tag="hpp")
                for k in range(CH):
                    ks = slice(k * K_PER, (k + 1) * K_PER)
                    for t in range(2):
                        nc.tensor.matmul(
                            ast_ps[t][:, :], p_sb[:CP, ks],
                            a_sb[:CP, k * N_PER + t * 500: k * N_PER + (t + 1) * 500],
                            start=(k == 0), stop=(k == CH - 1))
                    nc.tensor.matmul(hp_ps[:, :], p_sb[:CP, ks], feat_sb[:CP, k * D:(k + 1) * D],
                                     start=(k == 0), stop=(k == CH - 1))
                for t in range(2):
                    nc.vector.tensor_copy(ast_sb[:, t * 500:(t + 1) * 500], ast_ps[t][:, :])
                # transpose a_sT back to node-major
                for k in range(CH):
                    asp = ps3.tile([CP, K_PER], F32, name=f"asp_{j}_{k}", tag="asp", bufs=2)
                    nc.tensor.transpose(asp[:, :], ast_sb[:K_PER, k * CP:(k + 1) * CP],
                                        id_sb[:K_PER, :K_PER])
                    nc.vector.tensor_copy(as_sb[:CP, k * K_PER:(k + 1) * K_PER], asp[:, :])
                adj_ps = ps3.tile([K_PER, K_PER], F32, name=f"adjp_{j}", tag="adjp")
                for k in range(CH):
                    ks = slice(k * K_PER, (k + 1) * K_PER)
                    nc.tensor.matmul(adj_ps[:, :], p_sb[:CP, ks], as_sb[:CP, ks],
                                     start=(k == 0), stop=(k == CH - 1))
                nc.vector.tensor_copy(adj_sb[:, :], adj_ps[:, :])
                nc.vector.tensor_copy(hp_sb[:, :], hp_ps[:, :])
            nc.sync.dma_start(dram["adj_out"][j], adj_sb[:, :])
            nc.sync.dma_start(dram["hp_out"][j], hp_sb[:, :])


def _build(with_bias: bool, phases: int = 3):
    key = ("mod", with_bias, phases)
    if key in _CACHE:
        return _CACHE[key]
    nc = bacc.Bacc("TRN2", target_bir_lowering=False, debug=False,
                   enable_asserts=False, num_devices=NCORES)
    dram = {
        "af": nc.dram_tensor("af", [GPC, CH, CP, N_PER], F32, kind="ExternalInput").ap(),
        "h_in": nc.dram_tensor("h_in", [GPC, CH, CP, D], F32, kind="ExternalInput").ap(),
        "icnt": nc.dram_tensor("icnt", [GPC, 128, N_PER], F32, kind="ExternalInput").ap(),
        "wf": nc.dram_tensor("wf", [256, D], F32, kind="ExternalInput").ap(),
        "gm": nc.dram_tensor("gm", [256, 256], F32, kind="ExternalInput").ap(),
        "wpb": nc.dram_tensor("wpb", [GPC, 256, K_PER], F32, kind="ExternalInput").ap(),
        "ident": nc.dram_tensor("ident", [128, 128], F32, kind="ExternalInput").ap(),
        "adj_out": nc.dram_tensor("adj_out", [GPC, K_PER, K_PER], F32, kind="ExternalOutput").ap(),
        "hp_out": nc.dram_tensor("hp_out", [GPC, K_PER, D], F32, kind="ExternalOutput").ap(),
        "b2": _CACHE.get("b2", 0.0),
    }
    if with_bias:
        dram["bvec"] = nc.dram_tensor("bvec", [1, D + 256], F32, kind="ExternalInput").ap()
        dram["bpb"] = nc.dram_tensor("bpb", [GPC, 1, K_PER], F32, kind="ExternalInput").ap()
    with tile.TileContext(nc) as tc:
        _emit(tc, dram, with_bias, phases)
    nc.compile()
    _CACHE[key] = nc
    return nc


def _host_prep(h, W_feat, b_feat, W_pool, b_pool, src, dst):
    h = np.asarray(h, np.float32)
    W_feat = np.asarray(W_feat, np.float32)
    W_pool = np.asarray(W_pool, np.float32)
    src = np.asarray(src, np.int64)
    dst = np.asarray(dst, np.int64)

    # dense per-graph adjacency counts M[g, s_local, d_local]
    idx = src * N_PER + (dst % N_PER)
    mcnt = np.bincount(idx, minlength=B * N_PER * N_PER)
    af = mcnt.astype(np.float32).reshape(B, CH, CP, N_PER)
    cnt = np.bincount(dst, minlength=N).astype(np.float32)
    icnt = (1.0 / np.maximum(cnt, 1.0)).astype(np.float32).reshape(B, 1, N_PER)
    icnt = np.ascontiguousarray(np.broadcast_to(icnt, (B, 128, N_PER)))

    gm = (W_pool.astype(np.float64) @ W_pool.astype(np.float64).T).astype(np.float32)
    h_in = h.reshape(B, CH, CP, D)
    wpb = np.ascontiguousarray(
        W_pool.reshape(256, B, K_PER).transpose(1, 0, 2))

    pad = GPC * NCORES - B
    af = np.concatenate([af, np.zeros((pad,) + af.shape[1:], af.dtype)])
    h_in = np.concatenate([h_in, np.zeros((pad,) + h_in.shape[1:], h_in.dtype)])
    icnt = np.concatenate([icnt, np.ones((pad,) + icnt.shape[1:], icnt.dtype)])
    wpb = np.concatenate([wpb, np.zeros((pad,) + wpb.shape[1:], wpb.dtype)])
    return af, h_in, icnt, gm, wpb


def kernel(h, W_feat, b_feat, W_pool, b_pool, src, dst, mask):
    adj_new, h_pool, _ = _run(h, W_feat, b_feat, W_pool, b_pool, src, dst, mask)
    return adj_new, h_pool


def _run(h, W_feat, b_feat, W_pool, b_pool, src, dst, mask, **run_kwargs):
    b_feat = np.asarray(b_feat, np.float32)
    b_pool = np.asarray(b_pool, np.float32)
    with_bias = bool(np.any(b_feat) or np.any(b_pool))
    af, h_in, icnt, gm, wpb = _host_prep(h, W_feat, b_feat, W_pool, b_pool, src, dst)
    ident = np.eye(128, dtype=np.float32)
    W_feat = np.asarray(W_feat, np.float32)

    if with_bias:
        u = 2.0 * (np.asarray(W_pool, np.float64) @ b_pool.astype(np.float64))
        bvec = np.concatenate([b_feat, u.astype(np.float32)]).reshape(1, -1).astype(np.float32)
        bpb = b_pool.reshape(1, B, K_PER).transpose(1, 0, 2)
        bpb = np.concatenate([bpb, np.zeros((GPC * NCORES - B, 1, K_PER), np.float32)])
        _CACHE["b2"] = float(b_pool.astype(np.float64) @ b_pool.astype(np.float64))

    nc = _build(with_bias)

    # slot (c, j) holds graph c + 8*j
    in_maps = []
    for c in range(NCORES):
        gsel = [c + NCORES * j for j in range(GPC)]
        m = {
            "af": np.ascontiguousarray(af[gsel]),
            "h_in": np.ascontiguousarray(h_in[gsel]),
            "icnt": np.ascontiguousarray(icnt[gsel]),
            "wf": W_feat,
            "gm": gm,
            "wpb": np.ascontiguousarray(wpb[gsel]),
            "ident": ident,
        }
        if with_bias:
            m["bvec"] = bvec
            m["bpb"] = np.ascontiguousarray(bpb[gsel])
        in_maps.append(m)

    res = bass_utils.run_bass_kernel_spmd(
        nc, in_maps, core_ids=list(range(NCORES)), **run_kwargs)

    adj_new = np.zeros((ASSIGN, ASSIGN), np.float32)
    h_pool = np.zeros((ASSIGN, D), np.float32)
    for c in range(NCORES):
        out = res.results[c]
        for j in range(GPC):
            g = c + NCORES * j
            if g >= B:
                continue
            s = slice(g * K_PER, (g + 1) * K_PER)
            adj_new[s, s] = out["adj_out"][j]
            h_pool[s] = out["hp_out"][j]
    return adj_new, h_pool, res
